# revision 11
# baseline (speedup 1.0000x reference)
"""TRN2 Bass kernel v3: PVT cross-attention, all-bf16.

Host prep: x transposed to xT bf16 (no on-chip x transposes), y bf16,
weights bf16. On-chip: bf16 matmuls everywhere (1 cyc/row + cheap
ldweights), bf16 transposes for y, ACT-Identity denominator extraction
(psum base64 -> sbuf base0 remap), optional DVE int16-schraudolph exp
offload on selected mc chunks.
"""
import sys
sys.path.insert(0, "/opt/trn_rl_repo")
from contextlib import ExitStack

import numpy as np
import ml_dtypes

import concourse.bass as bass
import concourse.tile as tile
from concourse import bacc, mybir, masks

dt = mybir.dt
AF = mybir.ActivationFunctionType
ALU = mybir.AluOpType
f32 = dt.float32
f32r = dt.float32r
bf16 = dt.bfloat16
i16 = dt.int16

N = 4096
C = 512
CC = 512
NH = 8
HD = 64
INNER = 512
NK = 1024
EPS = 1e-5
NS = 8
SCALE = HD ** -0.5

LOG2E = 1.4426950408889634
SCH_A = SCALE * 128.0 * LOG2E
SCH_B = 127.0 * 128.0 - 5.5
DVE_EXP_MCS = (2, 5)        # mc chunks whose exp runs on DVE (schraudolph)


def ts(i, s):
    return bass.ts(i, s)


def build_core_program():
    nc = bacc.Bacc("TRN2", target_bir_lowering=False, debug=False)

    xT_d = nc.dram_tensor("xT", (C, N), bf16, kind="ExternalInput").ap()
    y_d = nc.dram_tensor("y", (4096, CC), bf16, kind="ExternalInput").ap()
    wq_d = nc.dram_tensor("wq", (C, INNER), bf16, kind="ExternalInput").ap()
    wk_d = nc.dram_tensor("wk", (CC, INNER), bf16, kind="ExternalInput").ap()
    wv_d = nc.dram_tensor("wv", (CC, INNER), bf16, kind="ExternalInput").ap()
    wp_d = nc.dram_tensor("wp", (INNER, C), bf16, kind="ExternalInput").ap()
    srw_d = nc.dram_tensor("srw", (2, 2, CC, CC), bf16, kind="ExternalInput").ap()
    bproj_d = nc.dram_tensor("bproj", (C,), f32, kind="ExternalInput").ap()
    gcross_d = nc.dram_tensor("g_cross", (CC,), f32, kind="ExternalInput").ap()
    bcross_d = nc.dram_tensor("b_cross", (CC,), f32, kind="ExternalInput").ap()
    srb_d = nc.dram_tensor("sr_b", (CC,), f32, kind="ExternalInput").ap()
    gsr_d = nc.dram_tensor("g_sr", (CC,), f32, kind="ExternalInput").ap()
    bsr_d = nc.dram_tensor("b_sr", (CC,), f32, kind="ExternalInput").ap()
    out_d = nc.dram_tensor("out", (N, C), f32, kind="ExternalOutput").ap()

    with tile.TileContext(nc) as tc, ExitStack() as octx:
        wpool = octx.enter_context(tc.tile_pool(name="weights", bufs=1))
        kvpool = octx.enter_context(tc.tile_pool(name="kv", bufs=1))
        ppool = octx.enter_context(tc.tile_pool(name="prep", bufs=1))
        bpool = octx.enter_context(tc.tile_pool(name="stageB", bufs=2))
        expool = octx.enter_context(tc.tile_pool(name="expool", bufs=1))
        ps = octx.enter_context(tc.tile_pool(name="ps", bufs=2, space="PSUM"))

        # ---- constants ---------------------------------------------------
        ident16 = wpool.tile([128, 128], bf16, tag="id16")
        masks.make_identity(nc, ident16[:])
        epst = wpool.tile([128, 1], f32, tag="eps")
        nc.vector.memset(epst[:], EPS)
        onesf = wpool.tile([128, 1], f32, tag="onesf")
        nc.vector.memset(onesf[:], 1.0)
        onesr = wpool.tile([128, 1], f32r, tag="onesr")
        nc.vector.tensor_copy(onesr[:], onesf[:])
        onesrow = wpool.tile([1, 128], f32r, tag="onesrow")
        nc.vector.tensor_copy(onesrow[:], onesf[0:1, 0:1].broadcast_to((1, 128)))

        wq = [wpool.tile([128, INNER], bf16, tag=f"wq{c}", name=f"wq{c}")
              for c in range(4)]
        wp = [wpool.tile([128, C], bf16, tag=f"wp{c}", name=f"wp{c}")
              for c in range(4)]

        def emit_wq_wp_dmas():
            for c in range(4):
                nc.sync.dma_start(wq[c][:], wq_d[ts(c, 128), :])
                nc.sync.dma_start(wp[c][:], wp_d[ts(c, 128), :])

        def chanvec(name, src):
            t = wpool.tile([128, 4], f32, tag=name, name=name)
            nc.sync.dma_start(t[:], src.rearrange("(c p) -> p c", p=128))
            return t

        gcross = chanvec("gcross", gcross_d)
        bcross = chanvec("bcross", bcross_d)
        gsr = chanvec("gsr", gsr_d)
        bsr = chanvec("bsr", bsr_d)
        srb = chanvec("srb", srb_d)

        bproj_row = wpool.tile([1, C], f32, tag="bprow")
        nc.sync.dma_start(bproj_row[:], bproj_d.rearrange("(a c) -> a c", a=1))
        bproj_b = wpool.tile([128, C], f32, tag="bpb")
        nc.gpsimd.partition_broadcast(bproj_b[:], bproj_row[:])

        # persistent context tensors
        kT = [kvpool.tile([128, NK], bf16, tag=f"kT{c}", name=f"kT{c}")
              for c in range(4)]
        # va[mc][p, (h, 65)]: cols 0-63 v, col 64 = 1.0 (denominator row)
        va = [kvpool.tile([128, NH, HD + 1], bf16, tag=f"va{m}",
                          name=f"va{m}") for m in range(8)]
        for m in range(8):
            nc.vector.memset(va[m][:], 1.0)

        # =================================================================
        # Stage B prep helpers
        # =================================================================
        def emit_prep_dma(ns):
            xts = []
            for c in range(4):
                xt = ppool.tile([128, 512], bf16, tag="xload", name="xload",
                                bufs=8)
                nc.sync.dma_start(xt[:], xT_d[ts(c, 128), ts(ns, 512)])
                xts.append(xt)
            return xts

        def emit_prep_q(xts, qT):
            for icn in range(4):
                pq = ps.tile([128, 512], f32, tag="mm", name="pq", bufs=1)
                for ci in range(4):
                    nc.tensor.matmul(pq[:], wq[ci][:, ts(icn, 128)],
                                     xts[ci][:], start=(ci == 0),
                                     stop=(ci == 3))
                nc.vector.tensor_copy(qT[icn][:], pq[:])

        def new_qT():
            return [ppool.tile([128, 512], bf16, tag=f"qT{c}", name=f"qT{c}",
                               bufs=2) for c in range(4)]

        def emit_proj_block(outT_src, ns_prev, qc):
            pf = ps.tile([128, 512], f32, tag="mm", name="pf", bufs=1)
            for icn in range(4):
                nc.tensor.matmul(pf[:], outT_src[icn][:, ts(qc, 128)],
                                 wp[icn][:], start=(icn == 0),
                                 stop=(icn == 3))
            fin = bpool.tile([128, C], f32, tag="fin", name="fin")
            nc.vector.tensor_tensor(fin[:], pf[:], bproj_b[:], op=ALU.add)
            nc.sync.dma_start(out_d[ts(ns_prev * 4 + qc, 128), :], fin[:])

        # =================================================================
        # Stage A: context prep (y -> LN -> conv -> LN -> kv)
        # =================================================================
        emit_wq_wp_dmas()
        with ExitStack() as actx:
            apool = actx.enter_context(tc.tile_pool(name="stageA", bufs=2))
            a1pool = actx.enter_context(tc.tile_pool(name="stageA1", bufs=1))

            wk = [a1pool.tile([128, INNER], bf16, tag=f"wk{c}", name=f"wk{c}")
                  for c in range(4)]
            wv = [a1pool.tile([128, INNER], bf16, tag=f"wv{c}", name=f"wv{c}")
                  for c in range(4)]
            srw = {}
            for di in range(2):
                for dj in range(2):
                    for ci in range(4):
                        srw[(di, dj, ci)] = a1pool.tile(
                            [128, CC], bf16, tag=f"srw{di}{dj}{ci}",
                            name=f"srw{di}{dj}{ci}")

            def emit_wkv_dmas():
                for c in range(4):
                    nc.sync.dma_start(wk[c][:], wk_d[ts(c, 128), :])
                    nc.sync.dma_start(wv[c][:], wv_d[ts(c, 128), :])

            def emit_srw_dmas():
                for di in range(2):
                    for dj in range(2):
                        for ci in range(4):
                            nc.sync.dma_start(srw[(di, dj, ci)][:],
                                              srw_d[di, dj, ts(ci, 128), :])

            x_raw = [a1pool.tile([128, NK], f32r, tag=f"xr{c}", name=f"xr{c}")
                     for c in range(4)]
            xln = [a1pool.tile([128, NK], bf16, tag=f"xl{c}", name=f"xl{c}")
                   for c in range(4)]

            # ---- A1+A2 fused per output-row group ------------------------
            for gg in range(4):
                ytg = [apool.tile([128, 1024], bf16, tag=f"ytg{c}",
                                  name=f"ytg{c}", bufs=2) for c in range(4)]
                for t8 in range(8):
                    trow = gg * 8 + t8
                    yt = apool.tile([128, CC], bf16, tag="yload", bufs=4)
                    nc.sync.dma_start(yt[:], y_d[ts(trow, 128), :])
                    st = apool.tile([128, 6], f32, tag="bnst", bufs=4)
                    ag = apool.tile([128, 2], f32, tag="bnag", bufs=4)
                    nc.vector.bn_stats(st[:], yt[:])
                    nc.vector.bn_aggr(ag[:], st[:])
                    rstd = apool.tile([128, 1], f32, tag="rstd", bufs=4)
                    nc.scalar.activation(rstd[:], ag[:, 1:2], AF.Sqrt,
                                         bias=epst[:, 0:1])
                    nc.vector.reciprocal_approx_fast(rstd[:], rstd[:])
                    nmr = apool.tile([128, 1], f32, tag="nmr", bufs=4)
                    nc.vector.tensor_scalar(nmr[:], ag[:, 0:1], rstd[:, 0:1],
                                            -1.0, op0=ALU.mult, op1=ALU.mult)
                    yln = apool.tile([128, CC], bf16, tag="yln", bufs=4)
                    nc.scalar.activation(yln[:], yt[:], AF.Identity,
                                         bias=nmr[:, 0:1], scale=rstd[:, 0:1])
                    for c in range(4):
                        pt = ps.tile([128, 128], bf16, tag="att", bufs=3)
                        nc.tensor.transpose(pt[:], yln[:, ts(c, 128)],
                                            ident16[:])
                        if c % 2 == 0:
                            nc.scalar.activation(
                                ytg[c][:, ts(t8, 128)], pt[:], AF.Identity,
                                bias=bcross[:, c:c + 1],
                                scale=gcross[:, c:c + 1])
                        else:
                            nc.vector.tensor_scalar(
                                ytg[c][:, ts(t8, 128)], pt[:],
                                gcross[:, c:c + 1], bcross[:, c:c + 1],
                                op0=ALU.mult, op1=ALU.add)

                if gg == 0:
                    emit_srw_dmas()
                elif gg == 1:
                    emit_wkv_dmas()

                for co in range(4):
                    pc = ps.tile([128, 256], f32, tag="att", bufs=3)
                    first = True
                    for ci in range(4):
                        view = ytg[ci][:].rearrange(
                            "p (i two j s) -> p i two j s",
                            i=8, two=2, j=32, s=2)
                        for di in range(2):
                            for dj in range(2):
                                g = view[:, :, di:di + 1, :, dj:dj + 1]
                                nc.tensor.matmul(
                                    pc[:],
                                    srw[(di, dj, ci)][:, ts(co, 128)],
                                    g,
                                    start=first,
                                    stop=(ci == 3 and di == 1 and dj == 1))
                                first = False
                    nc.vector.tensor_scalar(
                        x_raw[co][:, ts(gg, 256)], pc[:], srb[:, co:co + 1],
                        None, op0=ALU.add)

            # ---- A3: LN_sr over x_raw ------------------------------------
            ssum = apool.tile([1, NK], f32, tag="ssum", bufs=1)
            ssq = apool.tile([1, NK], f32, tag="ssq", bufs=1)
            for sp in range(2):
                p_sum = ps.tile([1, 512], f32, tag="att", bufs=3)
                p_sq = ps.tile([1, 512], f32, tag="att", bufs=3)
                for ci in range(4):
                    nc.tensor.matmul(p_sum[:], onesr[:],
                                     x_raw[ci][:, ts(sp, 512)],
                                     start=(ci == 0), stop=(ci == 3))
                for ci in range(4):
                    sq = apool.tile([128, 512], f32r, tag="sq", bufs=2)
                    nc.scalar.activation(sq[:], x_raw[ci][:, ts(sp, 512)],
                                         AF.Square)
                    nc.tensor.matmul(p_sq[:], onesr[:], sq[:],
                                     start=(ci == 0), stop=(ci == 3))
                nc.vector.tensor_copy(ssum[:, ts(sp, 512)], p_sum[:])
                nc.vector.tensor_copy(ssq[:, ts(sp, 512)], p_sq[:])

            sc1 = apool.tile([1, NK], f32, tag="sc1", bufs=1)
            nc.vector.tensor_scalar(ssum[:], ssum[:], 1.0 / CC, None,
                                    op0=ALU.mult)
            nc.vector.tensor_scalar(ssq[:], ssq[:], 1.0 / CC, None,
                                    op0=ALU.mult)
            nc.vector.tensor_tensor(sc1[:], ssum[:], ssum[:], op=ALU.mult)
            nc.vector.tensor_tensor(ssq[:], ssq[:], sc1[:], op=ALU.subtract)
            nc.scalar.activation(sc1[:], ssq[:], AF.Sqrt, bias=epst[0:1, 0:1])
            nc.vector.reciprocal_approx_fast(sc1[:], sc1[:])
            nc.vector.scalar_tensor_tensor(ssum[:], ssum[:], -1.0, sc1[:],
                                           op0=ALU.mult, op1=ALU.mult)
            rb = a1pool.tile([128, NK], f32, tag="rb")
            nb = a1pool.tile([128, NK], f32, tag="nb")
            nc.gpsimd.partition_broadcast(rb[:], sc1[:])
            nc.gpsimd.partition_broadcast(nb[:], ssum[:])
            for ci in range(4):
                tmp = apool.tile([128, NK], f32, tag="lnt", bufs=2)
                nc.vector.tensor_tensor(tmp[:], x_raw[ci][:], rb[:],
                                        op=ALU.mult)
                nc.vector.tensor_tensor(tmp[:], tmp[:], nb[:], op=ALU.add)
                nc.vector.tensor_scalar(xln[ci][:], tmp[:],
                                        gsr[:, ci:ci + 1], bsr[:, ci:ci + 1],
                                        op0=ALU.mult, op1=ALU.add)

            # ---- A4: kv projections --------------------------------------
            for icn in range(4):
                for msp in range(2):
                    pk = ps.tile([128, 512], f32, tag="att", bufs=3)
                    for ci in range(4):
                        nc.tensor.matmul(
                            pk[:], wk[ci][:, ts(icn, 128)],
                            xln[ci][:, ts(msp, 512)],
                            start=(ci == 0), stop=(ci == 3))
                    nc.vector.tensor_copy(kT[icn][:, ts(msp, 512)], pk[:])
            for mc in range(8):
                pv = ps.tile([128, 512], f32, tag="att", bufs=3)
                for ci in range(4):
                    nc.tensor.matmul(
                        pv[:], xln[ci][:, ts(mc, 128)], wv[ci][:],
                        start=(ci == 0), stop=(ci == 3))
                nc.vector.tensor_copy(
                    va[mc][:, :, 0:HD],
                    pv[:].rearrange("p (h e) -> p h e", e=HD))

        # =================================================================
        # Stage B (pipelined)
        # =================================================================
        xts0 = emit_prep_dma(0)
        qT0 = new_qT()
        emit_prep_q(xts0, qT0)
        qT_list = [None] * NS
        qT_list[0] = qT0
        outT_list = [None] * NS

        for ns in range(NS):
            qT = qT_list[ns]
            outT = [bpool.tile([128, 512], bf16, tag=f"oT{c}", name=f"oT{c}")
                    for c in range(4)]
            filler = []
            if ns >= 1:
                for qc in range(4):
                    filler.append(('proj', qc))
            if ns + 1 < NS:
                qTn = new_qT()
                qT_list[ns + 1] = qTn
                xts = emit_prep_dma(ns + 1)
                filler.append(('pq', 0))
            nf = len(filler)
            fidx = 0
            it = 0
            for hp in range(4):
                po = [ps.tile([HD + 1, 512], f32, tag="att", name=f"po{e_}",
                              bufs=3) for e_ in range(2)]
                for mc in range(8):
                    pss = ps.tile([128, 1024], f32, tag="sc", bufs=2)
                    for e in range(2):
                        nc.tensor.matmul(
                            pss[:, ts(e, 512)],
                            kT[hp][ts(e, 64), ts(mc, 128)],
                            qT[hp][ts(e, 64), :],
                            start=True, stop=True,
                            tile_position=(64 * e, 0))
                    ex = expool.tile([128, 1024], bf16, tag="ex", name="ex",
                                     bufs=4)
                    if mc in DVE_EXP_MCS:
                        nc.vector.tensor_scalar(
                            ex[:].bitcast(i16), pss[:], SCH_A, SCH_B,
                            op0=ALU.mult, op1=ALU.add)
                    else:
                        nc.scalar.activation(ex[:], pss[:], AF.Exp,
                                             scale=SCALE)
                    while fidx < nf and fidx * 32 <= it * nf:
                        kind, qc = filler[fidx]
                        if kind == 'proj':
                            emit_proj_block(outT_list[ns - 1], ns - 1, qc)
                        else:
                            emit_prep_q(xts, qTn)
                        fidx += 1
                    it += 1
                    for e in range(2):
                        h = 2 * hp + e
                        nc.tensor.matmul(
                            po[e][:], va[mc][:, h, :], ex[:, ts(e, 512)],
                            start=(mc == 0), stop=(mc == 7))
                for e in range(2):
                    # epilogue: DVE extracts den row from psum (ACT is the
                    # bottleneck engine), DVE recip, gpsimd broadcast,
                    # DVE mult into outT
                    den0 = bpool.tile([1, 512], f32, tag="den0")
                    nc.vector.tensor_copy(den0[:], po[e][64:65, :])
                    rec0 = bpool.tile([1, 512], f32, tag="rec0")
                    nc.vector.reciprocal_approx_fast(rec0[:], den0[:])
                    bc_sb = bpool.tile([64, 512], f32, tag="bcsb")
                    nc.gpsimd.partition_broadcast(bc_sb[:], rec0[:])
                    nc.vector.tensor_tensor(outT[hp][ts(e, 64), :],
                                            po[e][0:HD, :], bc_sb[:],
                                            op=ALU.mult)
            outT_list[ns] = outT
        for qc in range(4):
            emit_proj_block(outT_list[NS - 1], NS - 1, qc)

    nc.compile()
    return nc


# ---------------------------------------------------------------------------
# Host side
# ---------------------------------------------------------------------------
BF = ml_dtypes.bfloat16

_NC_CACHE = None


def _get_nc():
    global _NC_CACHE
    if _NC_CACHE is None:
        _NC_CACHE = build_core_program()
    return _NC_CACHE


def _prep_weights(inputs):
    Wkv = np.asarray(inputs["Wkv"], np.float32)
    shared = {
        "wq": np.asarray(inputs["Wq"], np.float32).astype(BF),
        "wk": np.ascontiguousarray(Wkv[:, :INNER]).astype(BF),
        "wv": np.ascontiguousarray(Wkv[:, INNER:]).astype(BF),
        "wp": np.asarray(inputs["Wproj"], np.float32).astype(BF),
        "srw": np.asarray(inputs["sr_w"], np.float32).astype(BF),
        "bproj": np.asarray(inputs["bproj"], np.float32),
        "g_cross": np.asarray(inputs["g_cross"], np.float32),
        "b_cross": np.asarray(inputs["b_cross"], np.float32),
        "sr_b": np.asarray(inputs["sr_b"], np.float32),
        "g_sr": np.asarray(inputs["g_sr"], np.float32),
        "b_sr": np.asarray(inputs["b_sr"], np.float32),
    }
    return shared


def _run(inputs, trace=False, trace_kwargs=None):
    from concourse.bass_utils import run_bass_kernel_spmd
    nc = _get_nc()
    shared = _prep_weights(inputs)
    x = np.asarray(inputs["x"], np.float32)
    y = np.asarray(inputs["y"], np.float32)
    n_cores = 8
    in_maps = []
    for b in range(n_cores):
        m = dict(shared)
        m["xT"] = np.ascontiguousarray(x[b].T).astype(BF)
        m["y"] = np.ascontiguousarray(y[b]).astype(BF)
        in_maps.append(m)
    kw = {}
    if trace:
        kw["trace"] = True
        if trace_kwargs:
            kw.update(trace_kwargs)
    res = run_bass_kernel_spmd(nc, in_maps, list(range(n_cores)), **kw)
    out = np.stack([res.results[i]["out"] for i in range(n_cores)], axis=0)
    return out, res


def kernel(**inputs):
    out, _ = _run(inputs)
    return out



# revision 18
# speedup vs baseline: 1.1889x; 1.1889x over previous
"""TRN2 Bass kernel v3: PVT cross-attention, all-bf16.

Host prep: x transposed to xT bf16 (no on-chip x transposes), y bf16,
weights bf16. On-chip: bf16 matmuls everywhere (1 cyc/row + cheap
ldweights), bf16 transposes for y, ACT-Identity denominator extraction
(psum base64 -> sbuf base0 remap), optional DVE int16-schraudolph exp
offload on selected mc chunks.
"""
import sys
sys.path.insert(0, "/opt/trn_rl_repo")
from contextlib import ExitStack

import numpy as np
import ml_dtypes

import concourse.bass as bass
import concourse.tile as tile
from concourse import bacc, mybir, masks

dt = mybir.dt
AF = mybir.ActivationFunctionType
ALU = mybir.AluOpType
f32 = dt.float32
f32r = dt.float32r
bf16 = dt.bfloat16
i16 = dt.int16

N = 4096
C = 512
CC = 512
NH = 8
HD = 64
INNER = 512
NK = 1024
EPS = 1e-5
NS = 8
SCALE = HD ** -0.5

LOG2E = 1.4426950408889634
SCH_A = SCALE * 128.0 * LOG2E
SCH_B = 127.0 * 128.0 - 5.5
DVE_EXP_MCS = (2, 5)        # mc chunks whose exp runs on DVE (schraudolph)


def ts(i, s):
    return bass.ts(i, s)


def build_core_program():
    nc = bacc.Bacc("TRN2", target_bir_lowering=False, debug=False)

    xT_d = nc.dram_tensor("xT", (C, N), bf16, kind="ExternalInput").ap()
    y_d = nc.dram_tensor("y", (4096, CC), bf16, kind="ExternalInput").ap()
    wq_d = nc.dram_tensor("wq", (C, INNER), bf16, kind="ExternalInput").ap()
    wk_d = nc.dram_tensor("wk", (CC, INNER), bf16, kind="ExternalInput").ap()
    wv_d = nc.dram_tensor("wv", (CC, INNER), bf16, kind="ExternalInput").ap()
    wp_d = nc.dram_tensor("wp", (INNER, C), bf16, kind="ExternalInput").ap()
    srw_d = nc.dram_tensor("srw", (2, 2, CC, CC), bf16, kind="ExternalInput").ap()
    bproj_d = nc.dram_tensor("bproj", (C,), f32, kind="ExternalInput").ap()
    gcross_d = nc.dram_tensor("g_cross", (CC,), f32, kind="ExternalInput").ap()
    bcross_d = nc.dram_tensor("b_cross", (CC,), f32, kind="ExternalInput").ap()
    srb_d = nc.dram_tensor("sr_b", (CC,), f32, kind="ExternalInput").ap()
    gsr_d = nc.dram_tensor("g_sr", (CC,), f32, kind="ExternalInput").ap()
    bsr_d = nc.dram_tensor("b_sr", (CC,), f32, kind="ExternalInput").ap()
    out_d = nc.dram_tensor("out", (N, C), f32, kind="ExternalOutput").ap()

    with tile.TileContext(nc) as tc, ExitStack() as octx:
        wpool = octx.enter_context(tc.tile_pool(name="weights", bufs=1))
        kvpool = octx.enter_context(tc.tile_pool(name="kv", bufs=1))
        ppool = octx.enter_context(tc.tile_pool(name="prep", bufs=1))
        bpool = octx.enter_context(tc.tile_pool(name="stageB", bufs=2))
        expool = octx.enter_context(tc.tile_pool(name="expool", bufs=1))
        ps = octx.enter_context(tc.tile_pool(name="ps", bufs=2, space="PSUM"))

        # ---- constants ---------------------------------------------------
        ident16 = wpool.tile([128, 128], bf16, tag="id16")
        masks.make_identity(nc, ident16[:])
        epst = wpool.tile([128, 1], f32, tag="eps")
        nc.vector.memset(epst[:], EPS)
        onesf = wpool.tile([128, 1], f32, tag="onesf")
        nc.vector.memset(onesf[:], 1.0)
        onesr = wpool.tile([128, 1], f32r, tag="onesr")
        nc.vector.tensor_copy(onesr[:], onesf[:])
        onesrow = wpool.tile([1, 128], f32r, tag="onesrow")
        nc.vector.tensor_copy(onesrow[:], onesf[0:1, 0:1].broadcast_to((1, 128)))

        wq = [wpool.tile([128, INNER], bf16, tag=f"wq{c}", name=f"wq{c}")
              for c in range(4)]
        wp = [wpool.tile([128, C], bf16, tag=f"wp{c}", name=f"wp{c}")
              for c in range(4)]

        def emit_wq_wp_dmas():
            for c in range(4):
                nc.sync.dma_start(wq[c][:], wq_d[ts(c, 128), :])
                nc.sync.dma_start(wp[c][:], wp_d[ts(c, 128), :])

        def chanvec(name, src):
            t = wpool.tile([128, 4], f32, tag=name, name=name)
            nc.sync.dma_start(t[:], src.rearrange("(c p) -> p c", p=128))
            return t

        gcross = chanvec("gcross", gcross_d)
        bcross = chanvec("bcross", bcross_d)
        gsr = chanvec("gsr", gsr_d)
        bsr = chanvec("bsr", bsr_d)
        srb = chanvec("srb", srb_d)

        bproj_row = wpool.tile([1, C], f32, tag="bprow")
        nc.sync.dma_start(bproj_row[:], bproj_d.rearrange("(a c) -> a c", a=1))
        bproj_b = wpool.tile([128, C], f32, tag="bpb")
        nc.gpsimd.partition_broadcast(bproj_b[:], bproj_row[:])

        # persistent context tensors
        kT = [kvpool.tile([128, NK], bf16, tag=f"kT{c}", name=f"kT{c}")
              for c in range(4)]
        # va[mc][p, (h, 97)]: cols 0-63 v, 64-95 junk, col 96 = 1.0.
        # Ones at col 96 puts the softmax denominator at psum row 96 (a
        # legal 32-aligned psum partition base) so DVE can reciprocal it
        # straight out of psum with no extract op.
        VW = 97
        va = [kvpool.tile([128, NH, VW], bf16, tag=f"va{m}",
                          name=f"va{m}") for m in range(8)]
        for m in range(8):
            nc.vector.memset(va[m][:], 1.0)

        # =================================================================
        # Stage B prep helpers
        # =================================================================
        def emit_prep_dma(ns):
            xts = []
            for c in range(4):
                xt = ppool.tile([128, 512], bf16, tag="xload", name="xload",
                                bufs=8)
                nc.sync.dma_start(xt[:], xT_d[ts(c, 128), ts(ns, 512)])
                xts.append(xt)
            return xts

        def emit_prep_q(xts, qT):
            for icn in range(4):
                pq = ps.tile([128, 512], f32, tag="mm", name="pq", bufs=1)
                for ci in range(4):
                    nc.tensor.matmul(pq[:], wq[ci][:, ts(icn, 128)],
                                     xts[ci][:], start=(ci == 0),
                                     stop=(ci == 3))
                nc.vector.tensor_copy(qT[icn][:], pq[:])

        def new_qT():
            return [ppool.tile([128, 512], bf16, tag=f"qT{c}", name=f"qT{c}",
                               bufs=2) for c in range(4)]

        def emit_proj_block(outT_src, ns_prev, qc):
            pf = ps.tile([128, 512], f32, tag="mm", name="pf", bufs=1)
            for icn in range(4):
                nc.tensor.matmul(pf[:], outT_src[icn][:, ts(qc, 128)],
                                 wp[icn][:], start=(icn == 0),
                                 stop=(icn == 3))
            fin = bpool.tile([128, C], f32, tag="fin", name="fin")
            nc.vector.tensor_tensor(fin[:], pf[:], bproj_b[:], op=ALU.add)
            nc.sync.dma_start(out_d[ts(ns_prev * 4 + qc, 128), :], fin[:])

        # =================================================================
        # Stage A: context prep (y -> LN -> conv -> LN -> kv)
        # =================================================================
        emit_wq_wp_dmas()
        with ExitStack() as actx:
            apool = actx.enter_context(tc.tile_pool(name="stageA", bufs=2))
            a1pool = actx.enter_context(tc.tile_pool(name="stageA1", bufs=1))

            wk = [a1pool.tile([128, INNER], bf16, tag=f"wk{c}", name=f"wk{c}")
                  for c in range(4)]
            wv = [a1pool.tile([128, INNER], bf16, tag=f"wv{c}", name=f"wv{c}")
                  for c in range(4)]
            srw = {}
            for di in range(2):
                for dj in range(2):
                    for ci in range(4):
                        srw[(di, dj, ci)] = a1pool.tile(
                            [128, CC], bf16, tag=f"srw{di}{dj}{ci}",
                            name=f"srw{di}{dj}{ci}")

            def emit_wkv_dmas():
                for c in range(4):
                    nc.sync.dma_start(wk[c][:], wk_d[ts(c, 128), :])
                    nc.sync.dma_start(wv[c][:], wv_d[ts(c, 128), :])

            def emit_srw_dmas():
                for di in range(2):
                    for dj in range(2):
                        for ci in range(4):
                            nc.sync.dma_start(srw[(di, dj, ci)][:],
                                              srw_d[di, dj, ts(ci, 128), :])

            x_raw = [a1pool.tile([128, NK], f32r, tag=f"xr{c}", name=f"xr{c}")
                     for c in range(4)]
            xln = [a1pool.tile([128, NK], bf16, tag=f"xl{c}", name=f"xl{c}")
                   for c in range(4)]

            # ---- A1+A2 fused per output-row group ------------------------
            for gg in range(4):
                ytg = [apool.tile([128, 1024], bf16, tag=f"ytg{c}",
                                  name=f"ytg{c}", bufs=2) for c in range(4)]
                for t8 in range(8):
                    trow = gg * 8 + t8
                    yt = apool.tile([128, CC], bf16, tag="yload", bufs=4)
                    nc.sync.dma_start(yt[:], y_d[ts(trow, 128), :])
                    st = apool.tile([128, 6], f32, tag="bnst", bufs=4)
                    ag = apool.tile([128, 2], f32, tag="bnag", bufs=4)
                    nc.vector.bn_stats(st[:], yt[:])
                    nc.vector.bn_aggr(ag[:], st[:])
                    rstd = apool.tile([128, 1], f32, tag="rstd", bufs=4)
                    nc.scalar.activation(rstd[:], ag[:, 1:2], AF.Sqrt,
                                         bias=epst[:, 0:1])
                    nc.vector.reciprocal_approx_fast(rstd[:], rstd[:])
                    nmr = apool.tile([128, 1], f32, tag="nmr", bufs=4)
                    nc.vector.tensor_scalar(nmr[:], ag[:, 0:1], rstd[:, 0:1],
                                            -1.0, op0=ALU.mult, op1=ALU.mult)
                    yln = apool.tile([128, CC], bf16, tag="yln", bufs=4)
                    nc.scalar.activation(yln[:], yt[:], AF.Identity,
                                         bias=nmr[:, 0:1], scale=rstd[:, 0:1])
                    for c in range(4):
                        pt = ps.tile([128, 128], bf16, tag="att", bufs=3)
                        nc.tensor.transpose(pt[:], yln[:, ts(c, 128)],
                                            ident16[:])
                        if c % 2 == 0:
                            nc.scalar.activation(
                                ytg[c][:, ts(t8, 128)], pt[:], AF.Identity,
                                bias=bcross[:, c:c + 1],
                                scale=gcross[:, c:c + 1])
                        else:
                            nc.vector.tensor_scalar(
                                ytg[c][:, ts(t8, 128)], pt[:],
                                gcross[:, c:c + 1], bcross[:, c:c + 1],
                                op0=ALU.mult, op1=ALU.add)

                if gg == 0:
                    emit_srw_dmas()
                elif gg == 1:
                    emit_wkv_dmas()

                for co in range(4):
                    pc = ps.tile([128, 256], f32, tag="att", bufs=3)
                    first = True
                    for ci in range(4):
                        view = ytg[ci][:].rearrange(
                            "p (i two j s) -> p i two j s",
                            i=8, two=2, j=32, s=2)
                        for di in range(2):
                            for dj in range(2):
                                g = view[:, :, di:di + 1, :, dj:dj + 1]
                                nc.tensor.matmul(
                                    pc[:],
                                    srw[(di, dj, ci)][:, ts(co, 128)],
                                    g,
                                    start=first,
                                    stop=(ci == 3 and di == 1 and dj == 1))
                                first = False
                    nc.vector.tensor_scalar(
                        x_raw[co][:, ts(gg, 256)], pc[:], srb[:, co:co + 1],
                        None, op0=ALU.add)

            # ---- A3: LN_sr over x_raw ------------------------------------
            ssum = apool.tile([1, NK], f32, tag="ssum", bufs=1)
            ssq = apool.tile([1, NK], f32, tag="ssq", bufs=1)
            for sp in range(2):
                p_sum = ps.tile([1, 512], f32, tag="att", bufs=3)
                p_sq = ps.tile([1, 512], f32, tag="att", bufs=3)
                for ci in range(4):
                    nc.tensor.matmul(p_sum[:], onesr[:],
                                     x_raw[ci][:, ts(sp, 512)],
                                     start=(ci == 0), stop=(ci == 3))
                for ci in range(4):
                    sq = apool.tile([128, 512], f32r, tag="sq", bufs=2)
                    nc.scalar.activation(sq[:], x_raw[ci][:, ts(sp, 512)],
                                         AF.Square)
                    nc.tensor.matmul(p_sq[:], onesr[:], sq[:],
                                     start=(ci == 0), stop=(ci == 3))
                nc.vector.tensor_copy(ssum[:, ts(sp, 512)], p_sum[:])
                nc.vector.tensor_copy(ssq[:, ts(sp, 512)], p_sq[:])

            sc1 = apool.tile([1, NK], f32, tag="sc1", bufs=1)
            nc.vector.tensor_scalar(ssum[:], ssum[:], 1.0 / CC, None,
                                    op0=ALU.mult)
            nc.vector.tensor_scalar(ssq[:], ssq[:], 1.0 / CC, None,
                                    op0=ALU.mult)
            nc.vector.tensor_tensor(sc1[:], ssum[:], ssum[:], op=ALU.mult)
            nc.vector.tensor_tensor(ssq[:], ssq[:], sc1[:], op=ALU.subtract)
            nc.scalar.activation(sc1[:], ssq[:], AF.Sqrt, bias=epst[0:1, 0:1])
            nc.vector.reciprocal_approx_fast(sc1[:], sc1[:])
            nc.vector.scalar_tensor_tensor(ssum[:], ssum[:], -1.0, sc1[:],
                                           op0=ALU.mult, op1=ALU.mult)
            rb = a1pool.tile([128, NK], f32, tag="rb")
            nb = a1pool.tile([128, NK], f32, tag="nb")
            nc.gpsimd.partition_broadcast(rb[:], sc1[:])
            nc.gpsimd.partition_broadcast(nb[:], ssum[:])
            for ci in range(4):
                tmp = apool.tile([128, NK], f32, tag="lnt", bufs=2)
                nc.vector.tensor_tensor(tmp[:], x_raw[ci][:], rb[:],
                                        op=ALU.mult)
                nc.vector.tensor_tensor(tmp[:], tmp[:], nb[:], op=ALU.add)
                nc.vector.tensor_scalar(xln[ci][:], tmp[:],
                                        gsr[:, ci:ci + 1], bsr[:, ci:ci + 1],
                                        op0=ALU.mult, op1=ALU.add)

            # ---- A4: kv projections --------------------------------------
            for icn in range(4):
                for msp in range(2):
                    pk = ps.tile([128, 512], f32, tag="att", bufs=3)
                    for ci in range(4):
                        nc.tensor.matmul(
                            pk[:], wk[ci][:, ts(icn, 128)],
                            xln[ci][:, ts(msp, 512)],
                            start=(ci == 0), stop=(ci == 3))
                    nc.vector.tensor_copy(kT[icn][:, ts(msp, 512)], pk[:])
            for mc in range(8):
                pv = ps.tile([128, 512], f32, tag="att", bufs=3)
                for ci in range(4):
                    nc.tensor.matmul(
                        pv[:], xln[ci][:, ts(mc, 128)], wv[ci][:],
                        start=(ci == 0), stop=(ci == 3))
                nc.vector.tensor_copy(
                    va[mc][:, :, 0:HD],
                    pv[:].rearrange("p (h e) -> p h e", e=HD))

        # =================================================================
        # Stage B (pipelined)
        # =================================================================
        xts0 = emit_prep_dma(0)
        qT0 = new_qT()
        emit_prep_q(xts0, qT0)
        qT_list = [None] * NS
        qT_list[0] = qT0
        outT_list = [None] * NS

        for ns in range(NS):
            qT = qT_list[ns]
            outT = [bpool.tile([128, 512], bf16, tag=f"oT{c}", name=f"oT{c}")
                    for c in range(4)]
            filler = []
            if ns >= 1:
                for qc in range(4):
                    filler.append(('proj', qc))
            if ns + 1 < NS:
                qTn = new_qT()
                qT_list[ns + 1] = qTn
                xts = emit_prep_dma(ns + 1)
                filler.append(('pq', 0))
            nf = len(filler)
            fidx = 0
            it = 0
            for hp in range(4):
                po = [ps.tile([VW, 512], f32, tag="att", name=f"po{e_}",
                              bufs=3) for e_ in range(2)]
                for mc in range(8):
                    pss = ps.tile([128, 1024], f32, tag="sc", bufs=2)
                    for e in range(2):
                        nc.tensor.matmul(
                            pss[:, ts(e, 512)],
                            kT[hp][ts(e, 64), ts(mc, 128)],
                            qT[hp][ts(e, 64), :],
                            start=True, stop=True,
                            tile_position=(64 * e, 0))
                    ex = expool.tile([128, 1024], bf16, tag="ex", name="ex",
                                     bufs=4)
                    if mc in DVE_EXP_MCS:
                        nc.vector.tensor_scalar(
                            ex[:].bitcast(i16), pss[:], SCH_A, SCH_B,
                            op0=ALU.mult, op1=ALU.add)
                    else:
                        nc.scalar.activation(ex[:], pss[:], AF.Exp,
                                             scale=SCALE)
                    while fidx < nf and fidx * 32 <= it * nf:
                        kind, qc = filler[fidx]
                        if kind == 'proj':
                            emit_proj_block(outT_list[ns - 1], ns - 1, qc)
                        else:
                            emit_prep_q(xts, qTn)
                        fidx += 1
                    it += 1
                    for e in range(2):
                        h = 2 * hp + e
                        nc.tensor.matmul(
                            po[e][:], va[mc][:, h, :], ex[:, ts(e, 512)],
                            start=(mc == 0), stop=(mc == 7))
                # epilogue: extract den rows (psum row 96) — e0 on ACT
                # (ScalarE has the fast psum port), e1 on DVE — into one
                # [2,512] tile, single batched DVE recip, then per-e
                # gpsimd broadcast + DVE mult into outT
                dens = [bpool.tile([1, 512], f32, tag=f"den{e_}",
                                   name=f"den{e_}") for e_ in range(2)]
                nc.scalar.activation(dens[0][:], po[0][96:97, :],
                                     AF.Identity)
                nc.vector.tensor_copy(dens[1][:], po[1][96:97, :])
                for e in range(2):
                    nc.vector.reciprocal_approx_fast(dens[e][:], dens[e][:])
                    bc_sb = bpool.tile([64, 512], f32, tag="bcsb")
                    nc.gpsimd.partition_broadcast(bc_sb[:], dens[e][:])
                    nc.vector.tensor_tensor(outT[hp][ts(e, 64), :],
                                            po[e][0:HD, :], bc_sb[:],
                                            op=ALU.mult)
            outT_list[ns] = outT
        for qc in range(4):
            emit_proj_block(outT_list[NS - 1], NS - 1, qc)

    nc.compile()
    return nc


# ---------------------------------------------------------------------------
# Host side
# ---------------------------------------------------------------------------
BF = ml_dtypes.bfloat16

_NC_CACHE = None


def _get_nc():
    global _NC_CACHE
    if _NC_CACHE is None:
        _NC_CACHE = build_core_program()
    return _NC_CACHE


def _prep_weights(inputs):
    Wkv = np.asarray(inputs["Wkv"], np.float32)
    shared = {
        "wq": np.asarray(inputs["Wq"], np.float32).astype(BF),
        "wk": np.ascontiguousarray(Wkv[:, :INNER]).astype(BF),
        "wv": np.ascontiguousarray(Wkv[:, INNER:]).astype(BF),
        "wp": np.asarray(inputs["Wproj"], np.float32).astype(BF),
        "srw": np.asarray(inputs["sr_w"], np.float32).astype(BF),
        "bproj": np.asarray(inputs["bproj"], np.float32),
        "g_cross": np.asarray(inputs["g_cross"], np.float32),
        "b_cross": np.asarray(inputs["b_cross"], np.float32),
        "sr_b": np.asarray(inputs["sr_b"], np.float32),
        "g_sr": np.asarray(inputs["g_sr"], np.float32),
        "b_sr": np.asarray(inputs["b_sr"], np.float32),
    }
    return shared


def _run(inputs, trace=False, trace_kwargs=None):
    from concourse.bass_utils import run_bass_kernel_spmd
    nc = _get_nc()
    shared = _prep_weights(inputs)
    x = np.asarray(inputs["x"], np.float32)
    y = np.asarray(inputs["y"], np.float32)
    n_cores = 8
    in_maps = []
    for b in range(n_cores):
        m = dict(shared)
        m["xT"] = np.ascontiguousarray(x[b].T).astype(BF)
        m["y"] = np.ascontiguousarray(y[b]).astype(BF)
        in_maps.append(m)
    kw = {}
    if trace:
        kw["trace"] = True
        if trace_kwargs:
            kw.update(trace_kwargs)
    res = run_bass_kernel_spmd(nc, in_maps, list(range(n_cores)), **kw)
    out = np.stack([res.results[i]["out"] for i in range(n_cores)], axis=0)
    return out, res


def kernel(**inputs):
    out, _ = _run(inputs)
    return out



# revision 19
# speedup vs baseline: 1.2531x; 1.0541x over previous
"""TRN2 Bass kernel v3: PVT cross-attention, all-bf16.

Host prep: x transposed to xT bf16 (no on-chip x transposes), y bf16,
weights bf16. On-chip: bf16 matmuls everywhere (1 cyc/row + cheap
ldweights), bf16 transposes for y, ACT-Identity denominator extraction
(psum base64 -> sbuf base0 remap), optional DVE int16-schraudolph exp
offload on selected mc chunks.
"""
import sys
sys.path.insert(0, "/opt/trn_rl_repo")
from contextlib import ExitStack

import numpy as np
import ml_dtypes

import concourse.bass as bass
import concourse.tile as tile
from concourse import bacc, mybir, masks

dt = mybir.dt
AF = mybir.ActivationFunctionType
ALU = mybir.AluOpType
f32 = dt.float32
f32r = dt.float32r
bf16 = dt.bfloat16
i16 = dt.int16

N = 4096
C = 512
CC = 512
NH = 8
HD = 64
INNER = 512
NK = 1024
EPS = 1e-5
NS = 8
SCALE = HD ** -0.5

LOG2E = 1.4426950408889634
SCH_A = SCALE * 128.0 * LOG2E
SCH_B = 127.0 * 128.0 - 5.5
DVE_EXP_MCS = (2, 5)        # mc chunks whose exp runs on DVE (schraudolph)


def ts(i, s):
    return bass.ts(i, s)


def build_core_program():
    nc = bacc.Bacc("TRN2", target_bir_lowering=False, debug=False)

    xT_d = nc.dram_tensor("xT", (C, N), bf16, kind="ExternalInput").ap()
    y_d = nc.dram_tensor("y", (4096, CC), bf16, kind="ExternalInput").ap()
    wq_d = nc.dram_tensor("wq", (C, INNER), bf16, kind="ExternalInput").ap()
    wk_d = nc.dram_tensor("wk", (CC, INNER), bf16, kind="ExternalInput").ap()
    wv_d = nc.dram_tensor("wv", (CC, INNER), bf16, kind="ExternalInput").ap()
    wp_d = nc.dram_tensor("wp", (INNER, C), bf16, kind="ExternalInput").ap()
    srw_d = nc.dram_tensor("srw", (2, 2, CC, CC), bf16, kind="ExternalInput").ap()
    bproj_d = nc.dram_tensor("bproj", (C,), f32, kind="ExternalInput").ap()
    gcross_d = nc.dram_tensor("g_cross", (CC,), f32, kind="ExternalInput").ap()
    bcross_d = nc.dram_tensor("b_cross", (CC,), f32, kind="ExternalInput").ap()
    srb_d = nc.dram_tensor("sr_b", (CC,), f32, kind="ExternalInput").ap()
    gsr_d = nc.dram_tensor("g_sr", (CC,), f32, kind="ExternalInput").ap()
    bsr_d = nc.dram_tensor("b_sr", (CC,), f32, kind="ExternalInput").ap()
    out_d = nc.dram_tensor("out", (N, C), f32, kind="ExternalOutput").ap()

    with tile.TileContext(nc) as tc, ExitStack() as octx:
        wpool = octx.enter_context(tc.tile_pool(name="weights", bufs=1))
        kvpool = octx.enter_context(tc.tile_pool(name="kv", bufs=1))
        ppool = octx.enter_context(tc.tile_pool(name="prep", bufs=1))
        bpool = octx.enter_context(tc.tile_pool(name="stageB", bufs=2))
        expool = octx.enter_context(tc.tile_pool(name="expool", bufs=1))
        ps = octx.enter_context(tc.tile_pool(name="ps", bufs=2, space="PSUM"))

        # ---- constants ---------------------------------------------------
        ident16 = wpool.tile([128, 128], bf16, tag="id16")
        masks.make_identity(nc, ident16[:])
        epst = wpool.tile([128, 1], f32, tag="eps")
        nc.vector.memset(epst[:], EPS)
        onesf = wpool.tile([128, 1], f32, tag="onesf")
        nc.vector.memset(onesf[:], 1.0)
        onesr = wpool.tile([128, 1], f32r, tag="onesr")
        nc.vector.tensor_copy(onesr[:], onesf[:])
        onesrow = wpool.tile([1, 128], f32r, tag="onesrow")
        nc.vector.tensor_copy(onesrow[:], onesf[0:1, 0:1].broadcast_to((1, 128)))

        wq = [wpool.tile([128, INNER], bf16, tag=f"wq{c}", name=f"wq{c}")
              for c in range(4)]
        wp = [wpool.tile([128, C], bf16, tag=f"wp{c}", name=f"wp{c}")
              for c in range(4)]

        def emit_wq_wp_dmas():
            for c in range(4):
                nc.sync.dma_start(wq[c][:], wq_d[ts(c, 128), :])
                nc.sync.dma_start(wp[c][:], wp_d[ts(c, 128), :])

        def chanvec(name, src):
            t = wpool.tile([128, 4], f32, tag=name, name=name)
            nc.sync.dma_start(t[:], src.rearrange("(c p) -> p c", p=128))
            return t

        gcross = chanvec("gcross", gcross_d)
        bcross = chanvec("bcross", bcross_d)
        gsr = chanvec("gsr", gsr_d)
        bsr = chanvec("bsr", bsr_d)
        srb = chanvec("srb", srb_d)

        bproj_row = wpool.tile([1, C], f32, tag="bprow")
        nc.sync.dma_start(bproj_row[:], bproj_d.rearrange("(a c) -> a c", a=1))
        bproj_b = wpool.tile([128, C], f32, tag="bpb")
        nc.gpsimd.partition_broadcast(bproj_b[:], bproj_row[:])

        # persistent context tensors
        kT = [kvpool.tile([128, NK], bf16, tag=f"kT{c}", name=f"kT{c}")
              for c in range(4)]
        # va[mc][p, (h, 97)]: cols 0-63 v, 64-95 junk, col 96 = 1.0.
        # Ones at col 96 puts the softmax denominator at psum row 96 (a
        # legal 32-aligned psum partition base) so DVE can reciprocal it
        # straight out of psum with no extract op.
        VW = 97
        va = [kvpool.tile([128, NH, VW], bf16, tag=f"va{m}",
                          name=f"va{m}") for m in range(8)]
        for m in range(8):
            nc.vector.memset(va[m][:], 1.0)

        # =================================================================
        # Stage B prep helpers
        # =================================================================
        def emit_prep_dma(ns):
            xts = []
            for c in range(4):
                xt = ppool.tile([128, 512], bf16, tag="xload", name="xload",
                                bufs=8)
                nc.sync.dma_start(xt[:], xT_d[ts(c, 128), ts(ns, 512)])
                xts.append(xt)
            return xts

        def emit_prep_q(xts, qT):
            for icn in range(4):
                pq = ps.tile([128, 512], f32, tag="mm", name="pq", bufs=1)
                for ci in range(4):
                    nc.tensor.matmul(pq[:], wq[ci][:, ts(icn, 128)],
                                     xts[ci][:], start=(ci == 0),
                                     stop=(ci == 3))
                nc.vector.tensor_copy(qT[icn][:], pq[:])

        def new_qT():
            return [ppool.tile([128, 512], bf16, tag=f"qT{c}", name=f"qT{c}",
                               bufs=2) for c in range(4)]

        def emit_proj_block(outT_src, ns_prev, qc):
            pf = ps.tile([128, 512], f32, tag="mm", name="pf", bufs=1)
            for icn in range(4):
                nc.tensor.matmul(pf[:], outT_src[icn][:, ts(qc, 128)],
                                 wp[icn][:], start=(icn == 0),
                                 stop=(icn == 3))
            fin = bpool.tile([128, C], f32, tag="fin", name="fin")
            nc.vector.tensor_tensor(fin[:], pf[:], bproj_b[:], op=ALU.add)
            nc.sync.dma_start(out_d[ts(ns_prev * 4 + qc, 128), :], fin[:])

        # =================================================================
        # Stage A: context prep (y -> LN -> conv -> LN -> kv)
        # =================================================================
        emit_wq_wp_dmas()
        with ExitStack() as actx:
            apool = actx.enter_context(tc.tile_pool(name="stageA", bufs=2))
            a1pool = actx.enter_context(tc.tile_pool(name="stageA1", bufs=1))

            wk = [a1pool.tile([128, INNER], bf16, tag=f"wk{c}", name=f"wk{c}")
                  for c in range(4)]
            wv = [a1pool.tile([128, INNER], bf16, tag=f"wv{c}", name=f"wv{c}")
                  for c in range(4)]
            srw = {}
            for di in range(2):
                for dj in range(2):
                    for ci in range(4):
                        srw[(di, dj, ci)] = a1pool.tile(
                            [128, CC], bf16, tag=f"srw{di}{dj}{ci}",
                            name=f"srw{di}{dj}{ci}")

            def emit_wkv_dmas():
                for c in range(4):
                    nc.sync.dma_start(wk[c][:], wk_d[ts(c, 128), :])
                    nc.sync.dma_start(wv[c][:], wv_d[ts(c, 128), :])

            def emit_srw_dmas():
                for di in range(2):
                    for dj in range(2):
                        for ci in range(4):
                            nc.sync.dma_start(srw[(di, dj, ci)][:],
                                              srw_d[di, dj, ts(ci, 128), :])

            x_raw = [a1pool.tile([128, NK], f32r, tag=f"xr{c}", name=f"xr{c}")
                     for c in range(4)]
            xln = [a1pool.tile([128, NK], bf16, tag=f"xl{c}", name=f"xl{c}")
                   for c in range(4)]

            # ---- A1+A2 fused per output-row group ------------------------
            for gg in range(4):
                ytg = [apool.tile([128, 1024], bf16, tag=f"ytg{c}",
                                  name=f"ytg{c}", bufs=2) for c in range(4)]
                for t8 in range(8):
                    trow = gg * 8 + t8
                    yt = apool.tile([128, CC], bf16, tag="yload", bufs=4)
                    nc.sync.dma_start(yt[:], y_d[ts(trow, 128), :])
                    st = apool.tile([128, 6], f32, tag="bnst", bufs=4)
                    ag = apool.tile([128, 2], f32, tag="bnag", bufs=4)
                    nc.vector.bn_stats(st[:], yt[:])
                    nc.vector.bn_aggr(ag[:], st[:])
                    rstd = apool.tile([128, 1], f32, tag="rstd", bufs=4)
                    nc.scalar.activation(rstd[:], ag[:, 1:2], AF.Sqrt,
                                         bias=epst[:, 0:1])
                    nc.vector.reciprocal_approx_fast(rstd[:], rstd[:])
                    nmr = apool.tile([128, 1], f32, tag="nmr", bufs=4)
                    nc.vector.tensor_scalar(nmr[:], ag[:, 0:1], rstd[:, 0:1],
                                            -1.0, op0=ALU.mult, op1=ALU.mult)
                    yln = apool.tile([128, CC], bf16, tag="yln", bufs=4)
                    nc.scalar.activation(yln[:], yt[:], AF.Identity,
                                         bias=nmr[:, 0:1], scale=rstd[:, 0:1])
                    for c in range(4):
                        pt = ps.tile([128, 128], bf16, tag="att", bufs=3)
                        nc.tensor.transpose(pt[:], yln[:, ts(c, 128)],
                                            ident16[:])
                        if c % 2 == 0:
                            nc.scalar.activation(
                                ytg[c][:, ts(t8, 128)], pt[:], AF.Identity,
                                bias=bcross[:, c:c + 1],
                                scale=gcross[:, c:c + 1])
                        else:
                            nc.vector.tensor_scalar(
                                ytg[c][:, ts(t8, 128)], pt[:],
                                gcross[:, c:c + 1], bcross[:, c:c + 1],
                                op0=ALU.mult, op1=ALU.add)

                if gg == 0:
                    emit_srw_dmas()
                elif gg == 1:
                    emit_wkv_dmas()

                for co in range(4):
                    pc = ps.tile([128, 256], f32, tag="att", bufs=3)
                    first = True
                    for ci in range(4):
                        view = ytg[ci][:].rearrange(
                            "p (i two j s) -> p i two j s",
                            i=8, two=2, j=32, s=2)
                        for di in range(2):
                            for dj in range(2):
                                g = view[:, :, di:di + 1, :, dj:dj + 1]
                                nc.tensor.matmul(
                                    pc[:],
                                    srw[(di, dj, ci)][:, ts(co, 128)],
                                    g,
                                    start=first,
                                    stop=(ci == 3 and di == 1 and dj == 1))
                                first = False
                    nc.vector.tensor_scalar(
                        x_raw[co][:, ts(gg, 256)], pc[:], srb[:, co:co + 1],
                        None, op0=ALU.add)

            # ---- A3: LN_sr over x_raw ------------------------------------
            ssum = apool.tile([1, NK], f32, tag="ssum", bufs=1)
            ssq = apool.tile([1, NK], f32, tag="ssq", bufs=1)
            for sp in range(2):
                p_sum = ps.tile([1, 512], f32, tag="att", bufs=3)
                p_sq = ps.tile([1, 512], f32, tag="att", bufs=3)
                for ci in range(4):
                    nc.tensor.matmul(p_sum[:], onesr[:],
                                     x_raw[ci][:, ts(sp, 512)],
                                     start=(ci == 0), stop=(ci == 3))
                for ci in range(4):
                    sq = apool.tile([128, 512], f32r, tag="sq", bufs=2)
                    nc.scalar.activation(sq[:], x_raw[ci][:, ts(sp, 512)],
                                         AF.Square)
                    nc.tensor.matmul(p_sq[:], onesr[:], sq[:],
                                     start=(ci == 0), stop=(ci == 3))
                nc.vector.tensor_copy(ssum[:, ts(sp, 512)], p_sum[:])
                nc.vector.tensor_copy(ssq[:, ts(sp, 512)], p_sq[:])

            sc1 = apool.tile([1, NK], f32, tag="sc1", bufs=1)
            nc.vector.tensor_scalar(ssum[:], ssum[:], 1.0 / CC, None,
                                    op0=ALU.mult)
            nc.vector.tensor_scalar(ssq[:], ssq[:], 1.0 / CC, None,
                                    op0=ALU.mult)
            nc.vector.tensor_tensor(sc1[:], ssum[:], ssum[:], op=ALU.mult)
            nc.vector.tensor_tensor(ssq[:], ssq[:], sc1[:], op=ALU.subtract)
            nc.scalar.activation(sc1[:], ssq[:], AF.Sqrt, bias=epst[0:1, 0:1])
            nc.vector.reciprocal_approx_fast(sc1[:], sc1[:])
            nc.vector.scalar_tensor_tensor(ssum[:], ssum[:], -1.0, sc1[:],
                                           op0=ALU.mult, op1=ALU.mult)
            rb = a1pool.tile([128, NK], f32, tag="rb")
            nb = a1pool.tile([128, NK], f32, tag="nb")
            nc.gpsimd.partition_broadcast(rb[:], sc1[:])
            nc.gpsimd.partition_broadcast(nb[:], ssum[:])
            for ci in range(4):
                tmp = apool.tile([128, NK], f32, tag="lnt", bufs=2)
                nc.vector.tensor_tensor(tmp[:], x_raw[ci][:], rb[:],
                                        op=ALU.mult)
                nc.vector.tensor_tensor(tmp[:], tmp[:], nb[:], op=ALU.add)
                nc.vector.tensor_scalar(xln[ci][:], tmp[:],
                                        gsr[:, ci:ci + 1], bsr[:, ci:ci + 1],
                                        op0=ALU.mult, op1=ALU.add)

            # ---- A4: kv projections --------------------------------------
            for icn in range(4):
                for msp in range(2):
                    pk = ps.tile([128, 512], f32, tag="att", bufs=3)
                    for ci in range(4):
                        nc.tensor.matmul(
                            pk[:], wk[ci][:, ts(icn, 128)],
                            xln[ci][:, ts(msp, 512)],
                            start=(ci == 0), stop=(ci == 3))
                    nc.vector.tensor_copy(kT[icn][:, ts(msp, 512)], pk[:])
            for mc in range(8):
                pv = ps.tile([128, 512], f32, tag="att", bufs=3)
                for ci in range(4):
                    nc.tensor.matmul(
                        pv[:], xln[ci][:, ts(mc, 128)], wv[ci][:],
                        start=(ci == 0), stop=(ci == 3))
                nc.vector.tensor_copy(
                    va[mc][:, :, 0:HD],
                    pv[:].rearrange("p (h e) -> p h e", e=HD))

        # =================================================================
        # Stage B (pipelined)
        # =================================================================
        xts0 = emit_prep_dma(0)
        qT0 = new_qT()
        emit_prep_q(xts0, qT0)
        qT_list = [None] * NS
        qT_list[0] = qT0
        outT_list = [None] * NS

        for ns in range(NS):
            qT = qT_list[ns]
            outT = [bpool.tile([128, 512], bf16, tag=f"oT{c}", name=f"oT{c}")
                    for c in range(4)]
            filler = []
            if ns >= 1:
                for qc in range(4):
                    filler.append(('proj', qc))
            if ns + 1 < NS:
                qTn = new_qT()
                qT_list[ns + 1] = qTn
                xts = emit_prep_dma(ns + 1)
                filler.append(('pq', 0))
            nf = len(filler)
            fidx = 0
            it = 0
            for hp in range(4):
                # Two-phase attn@v: phase 1 runs scores+exp for all mc and
                # accumulates head e0 only; e0's epilogue then starts
                # mid-hp (po[0] frees early), while phase 2 accumulates
                # head e1 from the retained ex tiles. This removes the
                # hp-boundary stall where the serial epilogue chain
                # (extract->recip->broadcast->mult) blocked every engine
                # FIFO before the next hp could start.
                po = [ps.tile([VW, 512], f32, tag="att", name=f"po{e_}",
                              bufs=3) for e_ in range(2)]
                exs = []
                for mc in range(8):
                    pss = ps.tile([128, 1024], f32, tag="sc", bufs=2)
                    for e in range(2):
                        nc.tensor.matmul(
                            pss[:, ts(e, 512)],
                            kT[hp][ts(e, 64), ts(mc, 128)],
                            qT[hp][ts(e, 64), :],
                            start=True, stop=True,
                            tile_position=(64 * e, 0))
                    ex = expool.tile([128, 1024], bf16, tag="ex", name="ex",
                                     bufs=8)
                    if mc in DVE_EXP_MCS:
                        nc.vector.tensor_scalar(
                            ex[:].bitcast(i16), pss[:], SCH_A, SCH_B,
                            op0=ALU.mult, op1=ALU.add)
                    else:
                        nc.scalar.activation(ex[:], pss[:], AF.Exp,
                                             scale=SCALE)
                    exs.append(ex)
                    while fidx < nf and fidx * 32 <= it * nf:
                        kind, qc = filler[fidx]
                        if kind == 'proj':
                            emit_proj_block(outT_list[ns - 1], ns - 1, qc)
                        else:
                            emit_prep_q(xts, qTn)
                        fidx += 1
                    it += 1
                    nc.tensor.matmul(
                        po[0][:], va[mc][:, 2 * hp, :], ex[:, 0:512],
                        start=(mc == 0), stop=(mc == 7))
                # epilogue e0: ACT extract sits right after exp(hp,7) in
                # the ACT queue; DVE recip, gpsimd broadcast
                den0 = bpool.tile([1, 512], f32, tag="den0")
                nc.scalar.activation(den0[:], po[0][96:97, :], AF.Identity)
                nc.vector.reciprocal_approx_fast(den0[:], den0[:])
                bc0 = bpool.tile([64, 512], f32, tag="bcsb")
                nc.gpsimd.partition_broadcast(bc0[:], den0[:])
                # phase 2: attn@v for head e1
                for mc in range(8):
                    nc.tensor.matmul(
                        po[1][:], va[mc][:, 2 * hp + 1, :],
                        exs[mc][:, ts(1, 512)],
                        start=(mc == 0), stop=(mc == 7))
                nc.vector.tensor_tensor(outT[hp][0:64, :], po[0][0:HD, :],
                                        bc0[:], op=ALU.mult)
                # epilogue e1 (extract on DVE)
                den1 = bpool.tile([1, 512], f32, tag="den1")
                nc.vector.tensor_copy(den1[:], po[1][96:97, :])
                nc.vector.reciprocal_approx_fast(den1[:], den1[:])
                bc1 = bpool.tile([64, 512], f32, tag="bcsb")
                nc.gpsimd.partition_broadcast(bc1[:], den1[:])
                nc.vector.tensor_tensor(outT[hp][64:128, :], po[1][0:HD, :],
                                        bc1[:], op=ALU.mult)
            outT_list[ns] = outT
        for qc in range(4):
            emit_proj_block(outT_list[NS - 1], NS - 1, qc)

    nc.compile()
    return nc


# ---------------------------------------------------------------------------
# Host side
# ---------------------------------------------------------------------------
BF = ml_dtypes.bfloat16

_NC_CACHE = None


def _get_nc():
    global _NC_CACHE
    if _NC_CACHE is None:
        _NC_CACHE = build_core_program()
    return _NC_CACHE


def _prep_weights(inputs):
    Wkv = np.asarray(inputs["Wkv"], np.float32)
    shared = {
        "wq": np.asarray(inputs["Wq"], np.float32).astype(BF),
        "wk": np.ascontiguousarray(Wkv[:, :INNER]).astype(BF),
        "wv": np.ascontiguousarray(Wkv[:, INNER:]).astype(BF),
        "wp": np.asarray(inputs["Wproj"], np.float32).astype(BF),
        "srw": np.asarray(inputs["sr_w"], np.float32).astype(BF),
        "bproj": np.asarray(inputs["bproj"], np.float32),
        "g_cross": np.asarray(inputs["g_cross"], np.float32),
        "b_cross": np.asarray(inputs["b_cross"], np.float32),
        "sr_b": np.asarray(inputs["sr_b"], np.float32),
        "g_sr": np.asarray(inputs["g_sr"], np.float32),
        "b_sr": np.asarray(inputs["b_sr"], np.float32),
    }
    return shared


def _run(inputs, trace=False, trace_kwargs=None):
    from concourse.bass_utils import run_bass_kernel_spmd
    nc = _get_nc()
    shared = _prep_weights(inputs)
    x = np.asarray(inputs["x"], np.float32)
    y = np.asarray(inputs["y"], np.float32)
    n_cores = 8
    in_maps = []
    for b in range(n_cores):
        m = dict(shared)
        m["xT"] = np.ascontiguousarray(x[b].T).astype(BF)
        m["y"] = np.ascontiguousarray(y[b]).astype(BF)
        in_maps.append(m)
    kw = {}
    if trace:
        kw["trace"] = True
        if trace_kwargs:
            kw.update(trace_kwargs)
    res = run_bass_kernel_spmd(nc, in_maps, list(range(n_cores)), **kw)
    out = np.stack([res.results[i]["out"] for i in range(n_cores)], axis=0)
    return out, res


def kernel(**inputs):
    out, _ = _run(inputs)
    return out



# revision 20
# speedup vs baseline: 1.2866x; 1.0267x over previous
"""TRN2 Bass kernel v3: PVT cross-attention, all-bf16.

Host prep: x transposed to xT bf16 (no on-chip x transposes), y bf16,
weights bf16. On-chip: bf16 matmuls everywhere (1 cyc/row + cheap
ldweights), bf16 transposes for y, ACT-Identity denominator extraction
(psum base64 -> sbuf base0 remap), optional DVE int16-schraudolph exp
offload on selected mc chunks.
"""
import sys
sys.path.insert(0, "/opt/trn_rl_repo")
from contextlib import ExitStack

import numpy as np
import ml_dtypes

import concourse.bass as bass
import concourse.tile as tile
from concourse import bacc, mybir, masks

dt = mybir.dt
AF = mybir.ActivationFunctionType
ALU = mybir.AluOpType
f32 = dt.float32
f32r = dt.float32r
bf16 = dt.bfloat16
i16 = dt.int16

N = 4096
C = 512
CC = 512
NH = 8
HD = 64
INNER = 512
NK = 1024
EPS = 1e-5
NS = 8
SCALE = HD ** -0.5

LOG2E = 1.4426950408889634
SCH_A = SCALE * 128.0 * LOG2E
SCH_B = 127.0 * 128.0 - 5.5
DVE_EXP_MCS = (2, 5)        # mc chunks whose exp runs on DVE (schraudolph)


def ts(i, s):
    return bass.ts(i, s)


def build_core_program():
    nc = bacc.Bacc("TRN2", target_bir_lowering=False, debug=False)

    xT_d = nc.dram_tensor("xT", (C, N), bf16, kind="ExternalInput").ap()
    y_d = nc.dram_tensor("y", (4096, CC), bf16, kind="ExternalInput").ap()
    wq_d = nc.dram_tensor("wq", (C, INNER), bf16, kind="ExternalInput").ap()
    wk_d = nc.dram_tensor("wk", (CC, INNER), bf16, kind="ExternalInput").ap()
    wv_d = nc.dram_tensor("wv", (CC, INNER), bf16, kind="ExternalInput").ap()
    wp_d = nc.dram_tensor("wp", (INNER, C), bf16, kind="ExternalInput").ap()
    srw_d = nc.dram_tensor("srw", (2, 2, CC, CC), bf16, kind="ExternalInput").ap()
    bproj_d = nc.dram_tensor("bproj", (C,), f32, kind="ExternalInput").ap()
    gcross_d = nc.dram_tensor("g_cross", (CC,), f32, kind="ExternalInput").ap()
    bcross_d = nc.dram_tensor("b_cross", (CC,), f32, kind="ExternalInput").ap()
    srb_d = nc.dram_tensor("sr_b", (CC,), f32, kind="ExternalInput").ap()
    gsr_d = nc.dram_tensor("g_sr", (CC,), f32, kind="ExternalInput").ap()
    bsr_d = nc.dram_tensor("b_sr", (CC,), f32, kind="ExternalInput").ap()
    out_d = nc.dram_tensor("out", (N, C), f32, kind="ExternalOutput").ap()

    with tile.TileContext(nc) as tc, ExitStack() as octx:
        wpool = octx.enter_context(tc.tile_pool(name="weights", bufs=1))
        kvpool = octx.enter_context(tc.tile_pool(name="kv", bufs=1))
        ppool = octx.enter_context(tc.tile_pool(name="prep", bufs=1))
        bpool = octx.enter_context(tc.tile_pool(name="stageB", bufs=2))
        expool = octx.enter_context(tc.tile_pool(name="expool", bufs=1))
        ps = octx.enter_context(tc.tile_pool(name="ps", bufs=2, space="PSUM"))

        # ---- constants ---------------------------------------------------
        ident16 = wpool.tile([128, 128], bf16, tag="id16")
        masks.make_identity(nc, ident16[:])
        epst = wpool.tile([128, 1], f32, tag="eps")
        nc.vector.memset(epst[:], EPS)
        onesf = wpool.tile([128, 1], f32, tag="onesf")
        nc.vector.memset(onesf[:], 1.0)
        onesr = wpool.tile([128, 1], f32r, tag="onesr")
        nc.vector.tensor_copy(onesr[:], onesf[:])
        onesrow = wpool.tile([1, 128], f32r, tag="onesrow")
        nc.vector.tensor_copy(onesrow[:], onesf[0:1, 0:1].broadcast_to((1, 128)))

        wq = [wpool.tile([128, INNER], bf16, tag=f"wq{c}", name=f"wq{c}")
              for c in range(4)]
        wp = [wpool.tile([128, C], bf16, tag=f"wp{c}", name=f"wp{c}")
              for c in range(4)]

        def emit_wq_wp_dmas():
            for c in range(4):
                nc.sync.dma_start(wq[c][:], wq_d[ts(c, 128), :])
                nc.sync.dma_start(wp[c][:], wp_d[ts(c, 128), :])

        def chanvec(name, src):
            t = wpool.tile([128, 4], f32, tag=name, name=name)
            nc.sync.dma_start(t[:], src.rearrange("(c p) -> p c", p=128))
            return t

        gcross = chanvec("gcross", gcross_d)
        bcross = chanvec("bcross", bcross_d)
        gsr = chanvec("gsr", gsr_d)
        bsr = chanvec("bsr", bsr_d)
        srb = chanvec("srb", srb_d)

        bproj_row = wpool.tile([1, C], f32, tag="bprow")
        nc.sync.dma_start(bproj_row[:], bproj_d.rearrange("(a c) -> a c", a=1))
        bproj_b = wpool.tile([128, C], f32, tag="bpb")
        nc.gpsimd.partition_broadcast(bproj_b[:], bproj_row[:])

        # persistent context tensors
        kT = [kvpool.tile([128, NK], bf16, tag=f"kT{c}", name=f"kT{c}")
              for c in range(4)]
        # va[mc][p, (h, 97)]: cols 0-63 v, 64-95 junk, col 96 = 1.0.
        # Ones at col 96 puts the softmax denominator at psum row 96 (a
        # legal 32-aligned psum partition base) so DVE can reciprocal it
        # straight out of psum with no extract op.
        VW = 97
        va = [kvpool.tile([128, NH, VW], bf16, tag=f"va{m}",
                          name=f"va{m}") for m in range(8)]
        for m in range(8):
            nc.vector.memset(va[m][:], 1.0)

        # =================================================================
        # Stage B prep helpers
        # =================================================================
        def emit_prep_dma(ns):
            xts = []
            for c in range(4):
                xt = ppool.tile([128, 512], bf16, tag="xload", name="xload",
                                bufs=8)
                nc.sync.dma_start(xt[:], xT_d[ts(c, 128), ts(ns, 512)])
                xts.append(xt)
            return xts

        def emit_prep_q(xts, qT):
            for icn in range(4):
                pq = ps.tile([128, 512], f32, tag="mm", name="pq", bufs=1)
                for ci in range(4):
                    nc.tensor.matmul(pq[:], wq[ci][:, ts(icn, 128)],
                                     xts[ci][:], start=(ci == 0),
                                     stop=(ci == 3))
                nc.vector.tensor_copy(qT[icn][:], pq[:])

        def new_qT():
            return [ppool.tile([128, 512], bf16, tag=f"qT{c}", name=f"qT{c}",
                               bufs=2) for c in range(4)]

        def emit_proj_block(outT_src, ns_prev, qc):
            pf = ps.tile([128, 512], f32, tag="mm", name="pf", bufs=1)
            for icn in range(4):
                nc.tensor.matmul(pf[:], outT_src[icn][:, ts(qc, 128)],
                                 wp[icn][:], start=(icn == 0),
                                 stop=(icn == 3))
            fin = bpool.tile([128, C], f32, tag="fin", name="fin")
            nc.vector.tensor_tensor(fin[:], pf[:], bproj_b[:], op=ALU.add)
            nc.sync.dma_start(out_d[ts(ns_prev * 4 + qc, 128), :], fin[:])

        # =================================================================
        # Stage A: context prep (y -> LN -> conv -> LN -> kv)
        # =================================================================
        emit_wq_wp_dmas()
        with ExitStack() as actx:
            apool = actx.enter_context(tc.tile_pool(name="stageA", bufs=2))
            a1pool = actx.enter_context(tc.tile_pool(name="stageA1", bufs=1))

            wk = [a1pool.tile([128, INNER], bf16, tag=f"wk{c}", name=f"wk{c}")
                  for c in range(4)]
            wv = [a1pool.tile([128, INNER], bf16, tag=f"wv{c}", name=f"wv{c}")
                  for c in range(4)]
            srw = {}
            for di in range(2):
                for dj in range(2):
                    for ci in range(4):
                        srw[(di, dj, ci)] = a1pool.tile(
                            [128, CC], bf16, tag=f"srw{di}{dj}{ci}",
                            name=f"srw{di}{dj}{ci}")

            def emit_wkv_dmas():
                for c in range(4):
                    nc.sync.dma_start(wk[c][:], wk_d[ts(c, 128), :])
                    nc.sync.dma_start(wv[c][:], wv_d[ts(c, 128), :])

            def emit_srw_dmas():
                for di in range(2):
                    for dj in range(2):
                        for ci in range(4):
                            nc.sync.dma_start(srw[(di, dj, ci)][:],
                                              srw_d[di, dj, ts(ci, 128), :])

            x_raw = [a1pool.tile([128, NK], f32r, tag=f"xr{c}", name=f"xr{c}")
                     for c in range(4)]
            xln = [a1pool.tile([128, NK], bf16, tag=f"xl{c}", name=f"xl{c}")
                   for c in range(4)]

            # ---- A1+A2 fused per output-row group ------------------------
            for gg in range(4):
                ytg = [apool.tile([128, 1024], bf16, tag=f"ytg{c}",
                                  name=f"ytg{c}", bufs=2) for c in range(4)]
                for t8 in range(8):
                    trow = gg * 8 + t8
                    yt = apool.tile([128, CC], bf16, tag="yload", bufs=4)
                    nc.sync.dma_start(yt[:], y_d[ts(trow, 128), :])
                    st = apool.tile([128, 6], f32, tag="bnst", bufs=4)
                    ag = apool.tile([128, 2], f32, tag="bnag", bufs=4)
                    nc.vector.bn_stats(st[:], yt[:])
                    nc.vector.bn_aggr(ag[:], st[:])
                    rstd = apool.tile([128, 1], f32, tag="rstd", bufs=4)
                    nc.scalar.activation(rstd[:], ag[:, 1:2], AF.Sqrt,
                                         bias=epst[:, 0:1])
                    nc.vector.reciprocal_approx_fast(rstd[:], rstd[:])
                    nmr = apool.tile([128, 1], f32, tag="nmr", bufs=4)
                    nc.vector.tensor_scalar(nmr[:], ag[:, 0:1], rstd[:, 0:1],
                                            -1.0, op0=ALU.mult, op1=ALU.mult)
                    yln = apool.tile([128, CC], bf16, tag="yln", bufs=4)
                    nc.scalar.activation(yln[:], yt[:], AF.Identity,
                                         bias=nmr[:, 0:1], scale=rstd[:, 0:1])
                    for c in range(4):
                        pt = ps.tile([128, 128], bf16, tag="att", bufs=3)
                        nc.tensor.transpose(pt[:], yln[:, ts(c, 128)],
                                            ident16[:])
                        if c % 2 == 0:
                            nc.scalar.activation(
                                ytg[c][:, ts(t8, 128)], pt[:], AF.Identity,
                                bias=bcross[:, c:c + 1],
                                scale=gcross[:, c:c + 1])
                        else:
                            nc.vector.tensor_scalar(
                                ytg[c][:, ts(t8, 128)], pt[:],
                                gcross[:, c:c + 1], bcross[:, c:c + 1],
                                op0=ALU.mult, op1=ALU.add)

                if gg == 0:
                    emit_srw_dmas()
                elif gg == 1:
                    emit_wkv_dmas()

                for co in range(4):
                    pc = ps.tile([128, 256], f32, tag="att", bufs=3)
                    first = True
                    for ci in range(4):
                        view = ytg[ci][:].rearrange(
                            "p (i two j s) -> p i two j s",
                            i=8, two=2, j=32, s=2)
                        for di in range(2):
                            for dj in range(2):
                                g = view[:, :, di:di + 1, :, dj:dj + 1]
                                nc.tensor.matmul(
                                    pc[:],
                                    srw[(di, dj, ci)][:, ts(co, 128)],
                                    g,
                                    start=first,
                                    stop=(ci == 3 and di == 1 and dj == 1))
                                first = False
                    nc.vector.tensor_scalar(
                        x_raw[co][:, ts(gg, 256)], pc[:], srb[:, co:co + 1],
                        None, op0=ALU.add)

            # ---- A3: LN_sr over x_raw ------------------------------------
            ssum = apool.tile([1, NK], f32, tag="ssum", bufs=1)
            ssq = apool.tile([1, NK], f32, tag="ssq", bufs=1)
            for sp in range(2):
                p_sum = ps.tile([1, 512], f32, tag="att", bufs=3)
                p_sq = ps.tile([1, 512], f32, tag="att", bufs=3)
                for ci in range(4):
                    nc.tensor.matmul(p_sum[:], onesr[:],
                                     x_raw[ci][:, ts(sp, 512)],
                                     start=(ci == 0), stop=(ci == 3))
                for ci in range(4):
                    sq = apool.tile([128, 512], f32r, tag="sq", bufs=2)
                    nc.scalar.activation(sq[:], x_raw[ci][:, ts(sp, 512)],
                                         AF.Square)
                    nc.tensor.matmul(p_sq[:], onesr[:], sq[:],
                                     start=(ci == 0), stop=(ci == 3))
                nc.vector.tensor_copy(ssum[:, ts(sp, 512)], p_sum[:])
                nc.vector.tensor_copy(ssq[:, ts(sp, 512)], p_sq[:])

            sc1 = apool.tile([1, NK], f32, tag="sc1", bufs=1)
            nc.vector.tensor_scalar(ssum[:], ssum[:], 1.0 / CC, None,
                                    op0=ALU.mult)
            nc.vector.tensor_scalar(ssq[:], ssq[:], 1.0 / CC, None,
                                    op0=ALU.mult)
            nc.vector.tensor_tensor(sc1[:], ssum[:], ssum[:], op=ALU.mult)
            nc.vector.tensor_tensor(ssq[:], ssq[:], sc1[:], op=ALU.subtract)
            nc.scalar.activation(sc1[:], ssq[:], AF.Sqrt, bias=epst[0:1, 0:1])
            nc.vector.reciprocal_approx_fast(sc1[:], sc1[:])
            nc.vector.scalar_tensor_tensor(ssum[:], ssum[:], -1.0, sc1[:],
                                           op0=ALU.mult, op1=ALU.mult)
            rb = a1pool.tile([128, NK], f32, tag="rb")
            nb = a1pool.tile([128, NK], f32, tag="nb")
            nc.gpsimd.partition_broadcast(rb[:], sc1[:])
            nc.gpsimd.partition_broadcast(nb[:], ssum[:])
            for ci in range(4):
                tmp = apool.tile([128, NK], f32, tag="lnt", bufs=2)
                nc.vector.tensor_tensor(tmp[:], x_raw[ci][:], rb[:],
                                        op=ALU.mult)
                nc.vector.tensor_tensor(tmp[:], tmp[:], nb[:], op=ALU.add)
                nc.vector.tensor_scalar(xln[ci][:], tmp[:],
                                        gsr[:, ci:ci + 1], bsr[:, ci:ci + 1],
                                        op0=ALU.mult, op1=ALU.add)

            # ---- A4: kv projections --------------------------------------
            for icn in range(4):
                for msp in range(2):
                    pk = ps.tile([128, 512], f32, tag="att", bufs=3)
                    for ci in range(4):
                        nc.tensor.matmul(
                            pk[:], wk[ci][:, ts(icn, 128)],
                            xln[ci][:, ts(msp, 512)],
                            start=(ci == 0), stop=(ci == 3))
                    nc.vector.tensor_copy(kT[icn][:, ts(msp, 512)], pk[:])
            for mc in range(8):
                pv = ps.tile([128, 512], f32, tag="att", bufs=3)
                for ci in range(4):
                    nc.tensor.matmul(
                        pv[:], xln[ci][:, ts(mc, 128)], wv[ci][:],
                        start=(ci == 0), stop=(ci == 3))
                nc.vector.tensor_copy(
                    va[mc][:, :, 0:HD],
                    pv[:].rearrange("p (h e) -> p h e", e=HD))

        # =================================================================
        # Stage B (pipelined)
        # =================================================================
        xts0 = emit_prep_dma(0)
        qT0 = new_qT()
        emit_prep_q(xts0, qT0)
        qT_list = [None] * NS
        qT_list[0] = qT0
        outT_list = [None] * NS

        for ns in range(NS):
            qT = qT_list[ns]
            outT = [bpool.tile([128, 512], bf16, tag=f"oT{c}", name=f"oT{c}")
                    for c in range(4)]
            filler = []
            if ns >= 1:
                for qc in range(4):
                    filler.append(('proj', qc))
            if ns + 1 < NS:
                qTn = new_qT()
                qT_list[ns + 1] = qTn
                xts = emit_prep_dma(ns + 1)
                filler.append(('pq', 0))
            nf = len(filler)
            fidx = 0
            it = 0
            # Software-pipelined attn@v: per hp, phase 1 runs scores+exp
            # and accumulates head e0; the e1 accumulation of the PREVIOUS
            # hp is interleaved into the same mc loop so the PE stream
            # never has a dead chain between hps and ACT never starves.
            # Epilogues run mid-stream: e0(hp) right after its last exp,
            # e1(hp-1) right after its last interleaved matmul.
            def emit_epi_e1(pv):
                po1_p, hp_p = pv
                den1 = bpool.tile([1, 512], f32, tag="den1")
                nc.vector.tensor_copy(den1[:], po1_p[96:97, :])
                nc.vector.reciprocal_approx_fast(den1[:], den1[:])
                bc1 = bpool.tile([64, 512], f32, tag="bcsb")
                nc.gpsimd.partition_broadcast(bc1[:], den1[:])
                nc.vector.tensor_tensor(outT[hp_p][64:128, :],
                                        po1_p[0:HD, :], bc1[:],
                                        op=ALU.mult)

            prev = None
            for hp in range(4):
                po = [ps.tile([VW, 512], f32, tag="att", name=f"po{e_}",
                              bufs=3) for e_ in range(2)]
                exs = []
                for mc in range(8):
                    pss = ps.tile([128, 1024], f32, tag="sc", bufs=2)
                    for e in range(2):
                        nc.tensor.matmul(
                            pss[:, ts(e, 512)],
                            kT[hp][ts(e, 64), ts(mc, 128)],
                            qT[hp][ts(e, 64), :],
                            start=True, stop=True,
                            tile_position=(64 * e, 0))
                    ex = expool.tile([128, 1024], bf16, tag="ex", name="ex",
                                     bufs=12)
                    if mc in DVE_EXP_MCS:
                        nc.vector.tensor_scalar(
                            ex[:].bitcast(i16), pss[:], SCH_A, SCH_B,
                            op0=ALU.mult, op1=ALU.add)
                    else:
                        nc.scalar.activation(ex[:], pss[:], AF.Exp,
                                             scale=SCALE)
                    exs.append(ex)
                    while fidx < nf and fidx * 32 <= it * nf:
                        kind, qc = filler[fidx]
                        if kind == 'proj':
                            emit_proj_block(outT_list[ns - 1], ns - 1, qc)
                        else:
                            emit_prep_q(xts, qTn)
                        fidx += 1
                    it += 1
                    nc.tensor.matmul(
                        po[0][:], va[mc][:, 2 * hp, :], ex[:, 0:512],
                        start=(mc == 0), stop=(mc == 7))
                    if prev is not None:
                        nc.tensor.matmul(
                            prev[0][:], va[mc][:, 2 * prev[2] + 1, :],
                            prev[1][mc][:, ts(1, 512)],
                            start=(mc == 0), stop=(mc == 7))
                # epilogue e0(hp): ACT extract sits right after exp(hp,7)
                den0 = bpool.tile([1, 512], f32, tag="den0")
                nc.scalar.activation(den0[:], po[0][96:97, :], AF.Identity)
                if prev is not None:
                    den1p = bpool.tile([1, 512], f32, tag="den1")
                    nc.vector.tensor_copy(den1p[:], prev[0][96:97, :])
                    nc.vector.reciprocal_approx_fast(den1p[:], den1p[:])
                    bc1p = bpool.tile([64, 512], f32, tag="bcsb")
                    nc.gpsimd.partition_broadcast(bc1p[:], den1p[:])
                nc.vector.reciprocal_approx_fast(den0[:], den0[:])
                bc0 = bpool.tile([64, 512], f32, tag="bcsb")
                nc.gpsimd.partition_broadcast(bc0[:], den0[:])
                if prev is not None:
                    nc.vector.tensor_tensor(outT[prev[2]][64:128, :],
                                            prev[0][0:HD, :], bc1p[:],
                                            op=ALU.mult)
                nc.vector.tensor_tensor(outT[hp][0:64, :], po[0][0:HD, :],
                                        bc0[:], op=ALU.mult)
                prev = (po[1], exs, hp)
            # tail: e1 chain of the last hp
            for mc in range(8):
                nc.tensor.matmul(
                    prev[0][:], va[mc][:, 2 * prev[2] + 1, :],
                    prev[1][mc][:, ts(1, 512)],
                    start=(mc == 0), stop=(mc == 7))
            emit_epi_e1((prev[0], prev[2]))
            outT_list[ns] = outT
        for qc in range(4):
            emit_proj_block(outT_list[NS - 1], NS - 1, qc)

    nc.compile()
    return nc


# ---------------------------------------------------------------------------
# Host side
# ---------------------------------------------------------------------------
BF = ml_dtypes.bfloat16

_NC_CACHE = None


def _get_nc():
    global _NC_CACHE
    if _NC_CACHE is None:
        _NC_CACHE = build_core_program()
    return _NC_CACHE


def _prep_weights(inputs):
    Wkv = np.asarray(inputs["Wkv"], np.float32)
    shared = {
        "wq": np.asarray(inputs["Wq"], np.float32).astype(BF),
        "wk": np.ascontiguousarray(Wkv[:, :INNER]).astype(BF),
        "wv": np.ascontiguousarray(Wkv[:, INNER:]).astype(BF),
        "wp": np.asarray(inputs["Wproj"], np.float32).astype(BF),
        "srw": np.asarray(inputs["sr_w"], np.float32).astype(BF),
        "bproj": np.asarray(inputs["bproj"], np.float32),
        "g_cross": np.asarray(inputs["g_cross"], np.float32),
        "b_cross": np.asarray(inputs["b_cross"], np.float32),
        "sr_b": np.asarray(inputs["sr_b"], np.float32),
        "g_sr": np.asarray(inputs["g_sr"], np.float32),
        "b_sr": np.asarray(inputs["b_sr"], np.float32),
    }
    return shared


def _run(inputs, trace=False, trace_kwargs=None):
    from concourse.bass_utils import run_bass_kernel_spmd
    nc = _get_nc()
    shared = _prep_weights(inputs)
    x = np.asarray(inputs["x"], np.float32)
    y = np.asarray(inputs["y"], np.float32)
    n_cores = 8
    in_maps = []
    for b in range(n_cores):
        m = dict(shared)
        m["xT"] = np.ascontiguousarray(x[b].T).astype(BF)
        m["y"] = np.ascontiguousarray(y[b]).astype(BF)
        in_maps.append(m)
    kw = {}
    if trace:
        kw["trace"] = True
        if trace_kwargs:
            kw.update(trace_kwargs)
    res = run_bass_kernel_spmd(nc, in_maps, list(range(n_cores)), **kw)
    out = np.stack([res.results[i]["out"] for i in range(n_cores)], axis=0)
    return out, res


def kernel(**inputs):
    out, _ = _run(inputs)
    return out

